# revision 1
# baseline (speedup 1.0000x reference)
"""OFT block-diagonal rotation forward (nn_Linear_12635793785535).

y = x @ blockdiag(rot_0..rot_63), rot_r = I + 2Q_r + 2Q_r^2 + 2Q_r^3 + 2Q_r^4
with Q_r the skew-symmetric matrix built from weight[r].

Sharding: data-parallel over tokens across 8 NeuronCores; the small derived
rotation blocks are replicated (per the problem's sharding hint).

Device kernel per core (1024 tokens, 4096 features):
  for each 128-token tile:
    DMA x tile [128, 4096] (natural layout, contiguous)
    for each of 32 feature pairs (2x64 blocks = 128 features):
      PE transpose x chunk -> PSUM [feat, tok]; DVE copy -> SBUF
      PE matmul: out[tok, feat] = xT.T @ rotpair (natural layout output)
      DVE/ACT copy PSUM -> y SBUF tile
    DMA y tile out
"""

import numpy as np

TOKENS = 8192
FEAT = 4096
R = 64
BLOCK = 64
NPAIR = 32  # pairs of 64-blocks -> 128-wide block-diagonal tiles
NUM_TERMS = 5
N_CORES = 8
TOK_SHARD = TOKENS // N_CORES  # 1024
TOK_TILE = 128
N_TTILES = TOK_SHARD // TOK_TILE  # 8

_CACHE = {}

# test.py can flip these before calling kernel()
TRACE = False
LAST_RESULTS = None


def _build_bass():
    from contextlib import ExitStack

    import concourse.tile as tile
    from concourse import bacc, mybir
    from concourse.masks import make_identity

    nc = bacc.Bacc(
        "TRN2",
        target_bir_lowering=False,
        debug=False,
        enable_asserts=False,
        num_devices=N_CORES,
    )
    x_d = nc.dram_tensor(
        "x", [TOK_SHARD, FEAT], mybir.dt.float32, kind="ExternalInput"
    ).ap()
    # rot layout [k=128, pair, c=128]: per-partition contiguous rows for DMA
    rot_d = nc.dram_tensor(
        "rot", [128, NPAIR, 128], mybir.dt.float32, kind="ExternalInput"
    ).ap()
    y_d = nc.dram_tensor(
        "y", [TOK_SHARD, FEAT], mybir.dt.float32, kind="ExternalOutput"
    ).ap()

    with tile.TileContext(nc) as tc, ExitStack() as ctx:
        const_pool = ctx.enter_context(tc.tile_pool(name="const", bufs=1))
        xpool = ctx.enter_context(tc.tile_pool(name="xin", bufs=2))
        ypool = ctx.enter_context(tc.tile_pool(name="yout", bufs=2))
        xtpool = ctx.enter_context(tc.tile_pool(name="xt", bufs=4))
        ps_t = ctx.enter_context(tc.tile_pool(name="ps_t", bufs=4, space="PSUM"))
        ps_y = ctx.enter_context(tc.tile_pool(name="ps_y", bufs=4, space="PSUM"))

        ident = const_pool.tile([128, 128], mybir.dt.float32)
        make_identity(nc, ident)

        rot_sb = const_pool.tile([128, NPAIR, 128], mybir.dt.float32)
        nc.sync.dma_start(rot_sb[:], rot_d)

        for t in range(N_TTILES):
            x_sb = xpool.tile([TOK_TILE, FEAT], mybir.dt.float32)
            nc.sync.dma_start(x_sb[:], x_d[t * TOK_TILE : (t + 1) * TOK_TILE, :])
            y_sb = ypool.tile([TOK_TILE, FEAT], mybir.dt.float32)
            for p in range(NPAIR):
                xt_ps = ps_t.tile([128, TOK_TILE], mybir.dt.float32)
                nc.tensor.transpose(
                    xt_ps[:], x_sb[:, p * 128 : (p + 1) * 128], ident[:]
                )
                xt_sb = xtpool.tile([128, TOK_TILE], mybir.dt.float32)
                nc.vector.tensor_copy(xt_sb[:], xt_ps[:])
                y_ps = ps_y.tile([TOK_TILE, 128], mybir.dt.float32)
                nc.tensor.matmul(
                    y_ps[:], xt_sb[:], rot_sb[:, p, :], start=True, stop=True
                )
                dst = y_sb[:, p * 128 : (p + 1) * 128]
                if p % 2 == 0:
                    nc.vector.tensor_copy(dst, y_ps[:])
                else:
                    nc.scalar.copy(dst, y_ps[:])
            nc.scalar.dma_start(y_d[t * TOK_TILE : (t + 1) * TOK_TILE, :], y_sb[:])

    nc.compile()
    return nc


def _host_rot_layout(weight):
    """Cayley-Neumann series on host (f32), packed as [k=128, pair, c=128]
    block-diagonal pair tiles (replicated across cores per sharding hint)."""
    w = np.asarray(weight, dtype=np.float32)
    rows, cols = np.triu_indices(BLOCK, k=1)
    Q = np.zeros((R, BLOCK, BLOCK), dtype=np.float32)
    Q[:, rows, cols] = w
    Q = Q - np.swapaxes(Q, 1, 2)
    eye = np.eye(BLOCK, dtype=np.float32)
    rot = eye[None, :, :] + 2.0 * Q
    Qp = Q
    for _ in range(2, NUM_TERMS):
        Qp = np.einsum("rij,rjk->rik", Qp, Q).astype(np.float32)
        rot = rot + 2.0 * Qp
    layout = np.zeros((128, NPAIR, 128), dtype=np.float32)
    for pair in range(NPAIR):
        layout[0:64, pair, 0:64] = rot[2 * pair]
        layout[64:128, pair, 64:128] = rot[2 * pair + 1]
    return layout


def kernel(x, weight):
    global LAST_RESULTS
    if "nc" not in _CACHE:
        _CACHE["nc"] = _build_bass()
    nc = _CACHE["nc"]

    from concourse.bass_utils import run_bass_kernel_spmd

    x = np.ascontiguousarray(np.asarray(x, dtype=np.float32))
    rot = _host_rot_layout(weight)
    in_maps = [
        {
            "x": np.ascontiguousarray(x[i * TOK_SHARD : (i + 1) * TOK_SHARD]),
            "rot": rot,
        }
        for i in range(N_CORES)
    ]
    res = run_bass_kernel_spmd(
        nc, in_maps, core_ids=list(range(N_CORES)), trace=TRACE
    )
    LAST_RESULTS = res
    out = np.concatenate([r["y"] for r in res.results], axis=0)
    return out


# revision 3
# speedup vs baseline: 1.3030x; 1.3030x over previous
"""OFT block-diagonal rotation forward (nn_Linear_12635793785535).

y = x @ blockdiag(rot_0..rot_63), rot_r = I + 2Q_r + 2Q_r^2 + 2Q_r^3 + 2Q_r^4
with Q_r the skew-symmetric matrix built from weight[r].

Sharding: data-parallel over tokens across 8 NeuronCores; the small derived
rotation blocks are replicated (per the problem's sharding hint).

Device kernel per core (1024 tokens, 4096 features):
  for each 128-token tile:
    DMA x tile [128, 4096] (natural layout, contiguous)
    for each group of 4 feature-pairs (512 features):
      4x PE transpose x chunks -> one PSUM bank [128, 512]; 1 DVE copy -> SBUF
      4x PE matmul out[tok, feat] = xT.T @ rotpair -> one PSUM bank [128, 512]
      1 copy PSUM -> y SBUF tile (alternating DVE / ACT)
    DMA y tile out
Transposes run as float32r (bit-preserving data movement, 1.5 vs 2 cyc/row).
"""

import numpy as np

TOKENS = 8192
FEAT = 4096
R = 64
BLOCK = 64
NPAIR = 32  # pairs of 64-blocks -> 128-wide block-diagonal tiles
GROUP = 4  # pairs per PSUM bank group (4 x 128 = 512 wide)
NGROUP = NPAIR // GROUP  # 8
NUM_TERMS = 5
N_CORES = 8
TOK_SHARD = TOKENS // N_CORES  # 1024
TOK_TILE = 128
N_TTILES = TOK_SHARD // TOK_TILE  # 8

F32R_TRANSPOSE = False

_CACHE = {}

# test.py can flip these before calling kernel()
TRACE = False
LAST_RESULTS = None


def _build_bass():
    from contextlib import ExitStack

    import concourse.tile as tile
    from concourse import bacc, mybir
    from concourse.masks import make_identity

    nc = bacc.Bacc(
        "TRN2",
        target_bir_lowering=False,
        debug=False,
        enable_asserts=False,
        num_devices=N_CORES,
    )
    x_d = nc.dram_tensor(
        "x", [TOK_SHARD, FEAT], mybir.dt.float32, kind="ExternalInput"
    ).ap()
    # rot layout [k=128, pair, c=128]: per-partition contiguous rows for DMA
    rot_d = nc.dram_tensor(
        "rot", [128, NPAIR, 128], mybir.dt.float32, kind="ExternalInput"
    ).ap()
    y_d = nc.dram_tensor(
        "y", [TOK_SHARD, FEAT], mybir.dt.float32, kind="ExternalOutput"
    ).ap()

    with tile.TileContext(nc) as tc, ExitStack() as ctx:
        const_pool = ctx.enter_context(tc.tile_pool(name="const", bufs=1))
        xpool = ctx.enter_context(tc.tile_pool(name="xin", bufs=2))
        ypool = ctx.enter_context(tc.tile_pool(name="yout", bufs=2))
        xtpool = ctx.enter_context(tc.tile_pool(name="xt", bufs=3))
        ps_t = ctx.enter_context(tc.tile_pool(name="ps_t", bufs=3, space="PSUM"))
        ps_y = ctx.enter_context(tc.tile_pool(name="ps_y", bufs=3, space="PSUM"))

        ident = const_pool.tile([128, 128], mybir.dt.float32)
        make_identity(nc, ident)

        rot_sb = const_pool.tile([128, NPAIR, 128], mybir.dt.float32)
        nc.sync.dma_start(rot_sb[:], rot_d)

        for t in range(N_TTILES):
            x_sb = xpool.tile([TOK_TILE, FEAT], mybir.dt.float32)
            nc.sync.dma_start(x_sb[:], x_d[t * TOK_TILE : (t + 1) * TOK_TILE, :])
            y_sb = ypool.tile([TOK_TILE, FEAT], mybir.dt.float32)
            for g in range(NGROUP):
                xt_ps = ps_t.tile([128, GROUP * TOK_TILE], mybir.dt.float32)
                for j in range(GROUP):
                    p = g * GROUP + j
                    src = x_sb[:, p * 128 : (p + 1) * 128]
                    dst = xt_ps[:, j * TOK_TILE : (j + 1) * TOK_TILE]
                    if F32R_TRANSPOSE:
                        nc.tensor.matmul(
                            dst.bitcast(mybir.dt.float32r),
                            src.bitcast(mybir.dt.float32r),
                            ident[:].bitcast(mybir.dt.float32r),
                            is_transpose=True,
                            start=True,
                            stop=True,
                        )
                    else:
                        nc.tensor.transpose(dst, src, ident[:])
                xt_sb = xtpool.tile([128, GROUP * TOK_TILE], mybir.dt.float32)
                nc.vector.tensor_copy(xt_sb[:], xt_ps[:])
                y_ps = ps_y.tile([TOK_TILE, GROUP * 128], mybir.dt.float32)
                for j in range(GROUP):
                    p = g * GROUP + j
                    nc.tensor.matmul(
                        y_ps[:, j * 128 : (j + 1) * 128],
                        xt_sb[:, j * TOK_TILE : (j + 1) * TOK_TILE],
                        rot_sb[:, p, :],
                        start=True,
                        stop=True,
                    )
                dst = y_sb[:, g * GROUP * 128 : (g + 1) * GROUP * 128]
                if g % 2 == 0:
                    nc.vector.tensor_copy(dst, y_ps[:])
                else:
                    nc.scalar.copy(dst, y_ps[:])
            nc.scalar.dma_start(y_d[t * TOK_TILE : (t + 1) * TOK_TILE, :], y_sb[:])

    nc.compile()
    return nc


def _host_rot_layout(weight):
    """Cayley-Neumann series on host (f32), packed as [k=128, pair, c=128]
    block-diagonal pair tiles (replicated across cores per sharding hint)."""
    w = np.asarray(weight, dtype=np.float32)
    rows, cols = np.triu_indices(BLOCK, k=1)
    Q = np.zeros((R, BLOCK, BLOCK), dtype=np.float32)
    Q[:, rows, cols] = w
    Q = Q - np.swapaxes(Q, 1, 2)
    eye = np.eye(BLOCK, dtype=np.float32)
    rot = eye[None, :, :] + 2.0 * Q
    Qp = Q
    for _ in range(2, NUM_TERMS):
        Qp = np.einsum("rij,rjk->rik", Qp, Q).astype(np.float32)
        rot = rot + 2.0 * Qp
    layout = np.zeros((128, NPAIR, 128), dtype=np.float32)
    for pair in range(NPAIR):
        layout[0:64, pair, 0:64] = rot[2 * pair]
        layout[64:128, pair, 64:128] = rot[2 * pair + 1]
    return layout


def kernel(x, weight):
    global LAST_RESULTS
    if "nc" not in _CACHE:
        _CACHE["nc"] = _build_bass()
    nc = _CACHE["nc"]

    from concourse.bass_utils import run_bass_kernel_spmd

    x = np.ascontiguousarray(np.asarray(x, dtype=np.float32))
    rot = _host_rot_layout(weight)
    in_maps = [
        {
            "x": np.ascontiguousarray(x[i * TOK_SHARD : (i + 1) * TOK_SHARD]),
            "rot": rot,
        }
        for i in range(N_CORES)
    ]
    res = run_bass_kernel_spmd(
        nc, in_maps, core_ids=list(range(N_CORES)), trace=TRACE
    )
    LAST_RESULTS = res
    out = np.concatenate([r["y"] for r in res.results], axis=0)
    return out


# revision 5
# speedup vs baseline: 1.3460x; 1.0330x over previous
"""OFT block-diagonal rotation forward (nn_Linear_12635793785535).

y = x @ blockdiag(rot_0..rot_63), rot_r = I + 2Q_r + 2Q_r^2 + 2Q_r^3 + 2Q_r^4
with Q_r the skew-symmetric matrix built from weight[r].

Sharding: data-parallel over tokens across 8 NeuronCores; the small derived
rotation blocks are replicated (per the problem's sharding hint).

Device kernel per core (1024 tokens, 4096 features):
  for each 128-token tile:
    DMA x tile [128, 4096] (natural layout, contiguous)
    for each group of 4 feature-pairs (512 features):
      4x PE transpose x chunks -> one PSUM bank [128, 512]; 1 DVE copy -> SBUF
      4x PE matmul out[tok, feat] = xT.T @ rotpair -> one PSUM bank [128, 512]
      1 copy PSUM -> y SBUF tile (alternating DVE / ACT)
    DMA y tile out
Transposes run as float32r (bit-preserving data movement, 1.5 vs 2 cyc/row).
"""

import numpy as np

TOKENS = 8192
FEAT = 4096
R = 64
BLOCK = 64
NPAIR = 32  # pairs of 64-blocks -> 128-wide block-diagonal tiles
GROUP = 4  # pairs per PSUM bank group (4 x 128 = 512 wide)
NGROUP = NPAIR // GROUP  # 8
NUM_TERMS = 5
N_CORES = 8
TOK_SHARD = TOKENS // N_CORES  # 1024
TOK_TILE = 128
N_TTILES = TOK_SHARD // TOK_TILE  # 8

F32R_TRANSPOSE = False

_CACHE = {}

# test.py can flip these before calling kernel()
TRACE = False
LAST_RESULTS = None


def _build_bass():
    from contextlib import ExitStack

    import concourse.tile as tile
    from concourse import bacc, mybir
    from concourse.masks import make_identity

    nc = bacc.Bacc(
        "TRN2",
        target_bir_lowering=False,
        debug=False,
        enable_asserts=False,
        num_devices=N_CORES,
    )
    x_d = nc.dram_tensor(
        "x", [TOK_SHARD, FEAT], mybir.dt.float32, kind="ExternalInput"
    ).ap()
    # rot layout [k=128, pair, c=128]: per-partition contiguous rows for DMA
    rot_d = nc.dram_tensor(
        "rot", [128, NPAIR, 128], mybir.dt.float32, kind="ExternalInput"
    ).ap()
    y_d = nc.dram_tensor(
        "y", [TOK_SHARD, FEAT], mybir.dt.float32, kind="ExternalOutput"
    ).ap()

    with tile.TileContext(nc) as tc, ExitStack() as ctx:
        const_pool = ctx.enter_context(tc.tile_pool(name="const", bufs=1))
        xpool = ctx.enter_context(tc.tile_pool(name="xin", bufs=3))
        ypool = ctx.enter_context(tc.tile_pool(name="yout", bufs=3))
        xtpool = ctx.enter_context(tc.tile_pool(name="xt", bufs=4))
        ps_t = ctx.enter_context(tc.tile_pool(name="ps_t", bufs=3, space="PSUM"))
        ps_y = ctx.enter_context(tc.tile_pool(name="ps_y", bufs=3, space="PSUM"))

        ident = const_pool.tile([128, 128], mybir.dt.float32)
        make_identity(nc, ident)

        rot_sb = const_pool.tile([128, NPAIR, 128], mybir.dt.float32)
        nc.sync.dma_start(rot_sb[:], rot_d)

        for t in range(N_TTILES):
            x_sb = xpool.tile([TOK_TILE, FEAT], mybir.dt.float32)
            nc.sync.dma_start(x_sb[:], x_d[t * TOK_TILE : (t + 1) * TOK_TILE, :])
            y_sb = ypool.tile([TOK_TILE, FEAT], mybir.dt.float32)
            for g in range(NGROUP):
                xt_ps = ps_t.tile([128, GROUP * TOK_TILE], mybir.dt.float32)
                for j in range(GROUP):
                    p = g * GROUP + j
                    src = x_sb[:, p * 128 : (p + 1) * 128]
                    dst = xt_ps[:, j * TOK_TILE : (j + 1) * TOK_TILE]
                    if F32R_TRANSPOSE:
                        nc.tensor.matmul(
                            dst.bitcast(mybir.dt.float32r),
                            src.bitcast(mybir.dt.float32r),
                            ident[:].bitcast(mybir.dt.float32r),
                            is_transpose=True,
                            start=True,
                            stop=True,
                        )
                    else:
                        nc.tensor.transpose(dst, src, ident[:])
                xt_sb = xtpool.tile([128, GROUP * TOK_TILE], mybir.dt.float32)
                nc.vector.tensor_copy(xt_sb[:], xt_ps[:])
                y_ps = ps_y.tile([TOK_TILE, GROUP * 128], mybir.dt.float32)
                for j in range(GROUP):
                    p = g * GROUP + j
                    nc.tensor.matmul(
                        y_ps[:, j * 128 : (j + 1) * 128],
                        xt_sb[:, j * TOK_TILE : (j + 1) * TOK_TILE],
                        rot_sb[:, p, :],
                        start=True,
                        stop=True,
                    )
                dst = y_sb[:, g * GROUP * 128 : (g + 1) * GROUP * 128]
                # ACT takes all y copies; DVE keeps the xT copies
                nc.scalar.copy(dst, y_ps[:])
            nc.scalar.dma_start(y_d[t * TOK_TILE : (t + 1) * TOK_TILE, :], y_sb[:])

    nc.compile()
    return nc


def _host_rot_layout(weight):
    """Cayley-Neumann series on host (f32), packed as [k=128, pair, c=128]
    block-diagonal pair tiles (replicated across cores per sharding hint)."""
    w = np.asarray(weight, dtype=np.float32)
    rows, cols = np.triu_indices(BLOCK, k=1)
    Q = np.zeros((R, BLOCK, BLOCK), dtype=np.float32)
    Q[:, rows, cols] = w
    Q = Q - np.swapaxes(Q, 1, 2)
    eye = np.eye(BLOCK, dtype=np.float32)
    rot = eye[None, :, :] + 2.0 * Q
    Qp = Q
    for _ in range(2, NUM_TERMS):
        Qp = np.einsum("rij,rjk->rik", Qp, Q).astype(np.float32)
        rot = rot + 2.0 * Qp
    layout = np.zeros((128, NPAIR, 128), dtype=np.float32)
    for pair in range(NPAIR):
        layout[0:64, pair, 0:64] = rot[2 * pair]
        layout[64:128, pair, 64:128] = rot[2 * pair + 1]
    return layout


def kernel(x, weight):
    global LAST_RESULTS
    if "nc" not in _CACHE:
        _CACHE["nc"] = _build_bass()
    nc = _CACHE["nc"]

    from concourse.bass_utils import run_bass_kernel_spmd

    x = np.ascontiguousarray(np.asarray(x, dtype=np.float32))
    rot = _host_rot_layout(weight)
    in_maps = [
        {
            "x": np.ascontiguousarray(x[i * TOK_SHARD : (i + 1) * TOK_SHARD]),
            "rot": rot,
        }
        for i in range(N_CORES)
    ]
    res = run_bass_kernel_spmd(
        nc, in_maps, core_ids=list(range(N_CORES)), trace=TRACE
    )
    LAST_RESULTS = res
    out = np.concatenate([r["y"] for r in res.results], axis=0)
    return out
